# revision 13
# baseline (speedup 1.0000x reference)
"""Loop-subdivision unpool (nn_LoopUnPool) on 8 Trainium2 NeuronCores.

Contract: kernel(pos, face_features, faces) -> (pos_out, faces_out, feats_out)
matching reference semantics:
  - unique sorted edges E (lexicographic, np.unique-compatible)
  - newV = 0.5*(pos[E0]+pos[E1]); pos_out = concat([pos, newV])
  - faces_out = 4 child-face blocks indexing original verts + (N + edge_id)
  - feats_out = concat([face_features]*4)

Sharding: faces (and face_features rows) are sharded across the 8 cores;
edge rows are sharded across the 8 cores; pos is sharded for the
passthrough copy. All heavy data movement (feature replication = ~97% of
bytes, faces_out construction, edge-midpoint averaging, pos copy) runs on
device. Host does the integer edge dedup (np.unique) and index prep.
"""

import numpy as np

NCORES = 8
P = 128
T_FEAT = 16384  # feature tile width (f32 per partition) -> 8MB tiles

_cache = {}


def _build_nc(Fs, R, Mnv, Mpp, N, C, reps=1, t_feat=None, feat_bufs=2):
    """Build the SPMD Bass program (identical on all 8 cores).

    Fs  : faces per core (padded)
    R   : faces rows per SBUF partition = Fs // 128
    Mnv : per-partition f32 count of the newV shard  (= 3*Es_pad//128)
    Mpp : per-partition f32 count of the pos shard   (= 3*Ns_pad//128)
    N   : number of original vertices (offset added to edge ids)
    C   : feature channels (128)
    """
    import concourse.tile as tile
    from concourse import bacc, mybir

    if t_feat is None:
        t_feat = T_FEAT
    nc = bacc.Bacc("TRN2", target_bir_lowering=False, debug=False,
                   num_devices=NCORES)

    Mf = Fs * C // P  # per-partition f32 count of the feature shard
    assert Fs % P == 0
    ff = nc.dram_tensor("ff", [P, Mf], mybir.dt.float32, kind="ExternalInput")
    fc = nc.dram_tensor("fc", [P, 3 * R], mybir.dt.int32, kind="ExternalInput")
    ee = nc.dram_tensor("ee", [P, 3 * R], mybir.dt.int32, kind="ExternalInput")
    p0 = nc.dram_tensor("p0", [P, Mnv], mybir.dt.float32, kind="ExternalInput")
    p1 = nc.dram_tensor("p1", [P, Mnv], mybir.dt.float32, kind="ExternalInput")
    pp = nc.dram_tensor("pp", [P, Mpp], mybir.dt.float32, kind="ExternalInput")

    of = nc.dram_tensor("of", [4, P, Mf], mybir.dt.float32, kind="ExternalOutput")
    ofc = nc.dram_tensor("ofc", [4, P, 3 * R], mybir.dt.int32, kind="ExternalOutput")
    onv = nc.dram_tensor("onv", [P, Mnv], mybir.dt.float32, kind="ExternalOutput")
    opp = nc.dram_tensor("opp", [P, Mpp], mybir.dt.float32, kind="ExternalOutput")

    with tile.TileContext(nc) as tc:
        with tc.tile_pool(name="misc", bufs=1) as mp, \
             tc.tile_pool(name="feat", bufs=feat_bufs) as fp:
          for _rep in range(reps):
            # ---- faces_out construction (strided int32 column ops) ----
            fct = mp.tile([P, 3 * R], mybir.dt.int32)
            eet = mp.tile([P, 3 * R], mybir.dt.int32)
            nc.sync.dma_start(fct[:], fc[:, :])
            nc.sync.dma_start(eet[:], ee[:, :])

            def col(t, j):
                # column j (of 3) of each packed (row,3) record in a tile
                return t[:].rearrange("p (r c) -> p r c", c=3)[:, :, j]

            # b0 = [f0, e0+N, e2+N]; b1 = [f1, e1+N, e0+N];
            # b2 = [f2, e2+N, e1+N]; b3 = [e1+N, e2+N, e0+N]
            specs = [
                (0, (0, 2)),
                (1, (1, 0)),
                (2, (2, 1)),
                (None, (1, 2, 0)),
            ]
            for j, (vcol, ecols) in enumerate(specs):
                bt = mp.tile([P, 3 * R], mybir.dt.int32, tag=f"b{j}")
                if vcol is not None:
                    nc.vector.tensor_copy(col(bt, 0), col(fct, vcol))
                    nc.vector.tensor_scalar_add(col(bt, 1), col(eet, ecols[0]), N)
                    nc.vector.tensor_scalar_add(col(bt, 2), col(eet, ecols[1]), N)
                else:
                    for s in range(3):
                        nc.vector.tensor_scalar_add(col(bt, s), col(eet, ecols[s]), N)
                nc.sync.dma_start(ofc[j, :, :], bt[:])

            # ---- newV = 0.5*(pos[E0] + pos[E1]) ----
            p0t = mp.tile([P, Mnv], mybir.dt.float32)
            p1t = mp.tile([P, Mnv], mybir.dt.float32)
            nc.sync.dma_start(p0t[:], p0[:, :])
            nc.sync.dma_start(p1t[:], p1[:, :])
            nc.vector.tensor_add(p0t[:], p0t[:], p1t[:])
            nc.vector.tensor_scalar_mul(p0t[:], p0t[:], 0.5)
            nc.sync.dma_start(onv[:, :], p0t[:])

            # ---- pos passthrough copy ----
            ppt = mp.tile([P, Mpp], mybir.dt.float32)
            nc.sync.dma_start(ppt[:], pp[:, :])
            nc.sync.dma_start(opp[:, :], ppt[:])

            # ---- feature replication x4 (the memory-dominant part) ----
            for t0 in range(0, Mf, t_feat):
                w = min(t_feat, Mf - t0)
                sl = slice(t0, t0 + w)
                tl = fp.tile([P, t_feat], mybir.dt.float32, tag="tl")
                nc.sync.dma_start(tl[:, :w], ff[:, sl])
                for j in range(4):
                    nc.sync.dma_start(of[j, :, sl], tl[:, :w])

    nc.finalize()
    return nc


def _pad_rows(a, rows):
    if a.shape[0] == rows:
        return np.ascontiguousarray(a)
    out = np.zeros((rows,) + a.shape[1:], dtype=a.dtype)
    out[:a.shape[0]] = a
    return out


def kernel(pos, face_features, faces):
    from concourse.bass_utils import run_bass_kernel_spmd

    pos = np.ascontiguousarray(np.asarray(pos), dtype=np.float32)
    ff = np.ascontiguousarray(np.asarray(face_features), dtype=np.float32)
    faces_np = np.asarray(faces)
    int_dtype = faces_np.dtype if np.issubdtype(faces_np.dtype, np.integer) \
        else np.int32
    N = pos.shape[0]
    F, C = ff.shape

    # ---------- host: edge dedup (np.unique-compatible) ----------
    f64 = faces_np.astype(np.int64)
    hE = np.concatenate([f64[:, [0, 1]], f64[:, [1, 2]], f64[:, [2, 0]]], axis=0)
    hE.sort(axis=1)
    shift = max(int(np.ceil(np.log2(max(int(N), 2)))), 1)
    keys = (hE[:, 0] << shift) | hE[:, 1]
    uk, hE2E = np.unique(keys, return_inverse=True)
    hE2E = hE2E.reshape(-1)
    nE = uk.shape[0]
    E0 = (uk >> shift).astype(np.int64)
    E1 = (uk & ((1 << shift) - 1)).astype(np.int64)

    posE0 = pos[E0]  # [nE, 3]
    posE1 = pos[E1]
    estack = np.stack([hE2E[:F], hE2E[F:2 * F], hE2E[2 * F:]], axis=1)
    estack = estack.astype(np.int32)
    fc32 = f64.astype(np.int32)

    # ---------- shard + pad ----------
    def ceil_to(x, m):
        return -(-x // m) * m

    Fs = ceil_to(-(-F // NCORES), P)          # faces per core
    R = Fs // P
    Es = ceil_to(-(-nE // NCORES), P)         # edges per core
    Ns = ceil_to(-(-N // NCORES), P)          # verts per core
    Mnv = 3 * Es // P
    Mpp = 3 * Ns // P
    Mf = Fs * C // P

    key = (Fs, R, Mnv, Mpp, N, C)
    if key not in _cache:
        _cache[key] = _build_nc(Fs, R, Mnv, Mpp, N, C)
    nc = _cache[key]

    in_maps = []
    for k in range(NCORES):
        fsl = slice(k * Fs, min((k + 1) * Fs, F))
        esl = slice(k * Es, min((k + 1) * Es, nE))
        nsl = slice(k * Ns, min((k + 1) * Ns, N))
        in_maps.append({
            "ff": _pad_rows(ff[fsl], Fs).reshape(P, Mf),
            "fc": _pad_rows(fc32[fsl], Fs).reshape(P, 3 * R),
            "ee": _pad_rows(estack[fsl], Fs).reshape(P, 3 * R),
            "p0": _pad_rows(posE0[esl], Es).reshape(P, Mnv),
            "p1": _pad_rows(posE1[esl], Es).reshape(P, Mnv),
            "pp": _pad_rows(pos[nsl], Ns).reshape(P, Mpp),
        })

    res = run_bass_kernel_spmd(nc, in_maps, core_ids=list(range(NCORES)))
    outs = res.results

    # ---------- host: reassemble ----------
    pos_out = np.empty((N + nE, 3), dtype=np.float32)
    faces_out = np.empty((4 * F, 3), dtype=int_dtype)
    feats_out = np.empty((4 * F, C), dtype=np.float32)
    for k in range(NCORES):
        o = outs[k]
        n0, n1 = k * Ns, min((k + 1) * Ns, N)
        if n0 < N:
            pos_out[n0:n1] = o["opp"].reshape(Ns, 3)[:n1 - n0]
        e0, e1 = k * Es, min((k + 1) * Es, nE)
        if e0 < nE:
            pos_out[N + e0:N + e1] = o["onv"].reshape(Es, 3)[:e1 - e0]
        f0, f1 = k * Fs, min((k + 1) * Fs, F)
        if f0 < F:
            nrow = f1 - f0
            for j in range(4):
                feats_out[j * F + f0:j * F + f1] = \
                    o["of"][j].reshape(Fs, C)[:nrow]
                faces_out[j * F + f0:j * F + f1] = \
                    o["ofc"][j].reshape(Fs, 3)[:nrow]
    return pos_out, faces_out, feats_out
